# revision 25
# baseline (speedup 1.0000x reference)
"""Causal multi-head self-attention with RoPE on 8 Trainium2 NeuronCores.

Sharding: data-parallel over batch (B=4 -> 2 cores per batch) x tensor-parallel
over heads (16 heads -> 8 per core). Each core computes q/k/v projections for
its 8 heads, RoPE, causal attention, and a partial o_proj; the host sums the
two partial o_proj outputs per batch.

v2: single fused pipeline. The ACT engine's exp is the attention bottleneck
(~1.15us per 128x1024 score tile), so projection / o_proj matmul chains are
emitted as paced "filler" units between attention matmuls: the PE never waits
for exp. All matmul operands are bf16 (inputs cast on host), PSUM stays f32;
exp writes bf16 e-tiles directly. Per-core layout as v1:
  - x^T and pre-transposed weight shards DMA'd in; Q^T/K^T head-major [dk,s];
    scores computed transposed [s_k,s_q]; ones-column-augmented V gives the
    softmax denominator for free in the AV accumulation.
  - Scores for the two heads of a 128-row chunk packed into PE row-groups via
    tile_position; one full-width exp covers both.
  - RoPE via evens/odds dk permutation (host-folded) + P_swap matmul;
    cos/sin tables built on device with Cody-Waite range reduction.
  - Diagonal-block causal mask via gpsimd affine_select; diag key-tiles run
    first in each accumulation group.
"""

import sys

sys.path.insert(0, "/opt/trn_rl_repo")

import numpy as np
import ml_dtypes

import concourse.bass as bass
import concourse.tile as tile
from concourse import bacc, mybir
from concourse.bass_utils import run_bass_kernel_spmd
from concourse.masks import make_identity

B, S, D, H = 4, 2048, 1024, 16
DK = D // H            # 64
HPC = H // 2           # 8 heads per core
DPC = HPC * DK         # 512 head dims per core
N_CORES = 8
HALF = DK // 2         # 32 rotary pairs
THETA = 10000.0

AF = mybir.ActivationFunctionType
F32 = mybir.dt.float32
F32R = mybir.dt.float32r
BF16 = mybir.dt.bfloat16
I32 = mybir.dt.int32

TWO_PI = 2.0 * np.pi
_CW_C1 = 6.28125
_CW_C2 = float(np.float32(9.67025756835937500e-4))
_CW_C3 = float(TWO_PI - _CW_C1 - np.float32(9.67025756835937500e-4))

# pacing model (ns) for the filler scheduler
_EXP_NS = 1210.0
_YIELD_NS = 450.0


def _mm_ns(n):
    return n / 2.4 + 90.0


def _build_program(debug=False):
    nc = bacc.Bacc("TRN2", target_bir_lowering=False, debug=False)

    xTc = nc.dram_tensor("xTc", [4, 128, D // 128, 512], BF16, kind="ExternalInput").ap()
    wqk = nc.dram_tensor("wqk", [128, 2, D // 128, DPC], BF16, kind="ExternalInput").ap()
    wvc = nc.dram_tensor("wvc", [128, D // 128, DPC], BF16, kind="ExternalInput").ap()
    woc = nc.dram_tensor("woc", [128, DPC // 128, D], BF16, kind="ExternalInput").ap()
    cst = nc.dram_tensor("cst", [128, S], F32, kind="ExternalInput").ap()
    sst = nc.dram_tensor("sst", [128, S], F32, kind="ExternalInput").ap()
    psw = nc.dram_tensor("psw", [128, 128], F32, kind="ExternalInput").ap()
    y = nc.dram_tensor("y", [S, D], F32, kind="ExternalOutput").ap()

    dbg = None
    if debug:
        dbg = {
            "cs_dump": nc.dram_tensor("cs_dump", [2, 128, S], F32, kind="ExternalOutput").ap(),
            "qk_dump": nc.dram_tensor("qk_dump", [128, 8, S], BF16, kind="ExternalOutput").ap(),
            "vp_dump": nc.dram_tensor("vp_dump", [128, S // 128, HPC * (DK + 1)], BF16, kind="ExternalOutput").ap(),
            "heads_dump": nc.dram_tensor("heads_dump", [128, DPC // 128, S], BF16, kind="ExternalOutput").ap(),
        }

    with tile.TileContext(nc) as tc:
        _emit(nc, tc, xTc, wqk, wvc, woc, cst, sst, psw, y, dbg)

    nc.compile()
    return nc


def _emit(nc, tc, xTc, wqk_in, wvc, woc, cst, sst, psw, y, dbg=None):
    import contextlib

    ctx = contextlib.ExitStack()
    with ctx:
        persist = ctx.enter_context(tc.tile_pool(name="persist", bufs=1))
        p_swap = persist.tile([128, 128], F32R)
        ones_col = persist.tile([128, 1], BF16)
        nc.vector.memset(ones_col, 1.0)
        ones_row = persist.tile([1, 64], BF16)
        nc.vector.memset(ones_row, 1.0)

        # ---- persistent tensors ----
        cs_pool = ctx.enter_context(tc.tile_pool(name="cs", bufs=1))
        cbig = cs_pool.tile([128, S], F32)
        sbig = cs_pool.tile([128, S], F32)
        qkT = ctx.enter_context(tc.tile_pool(name="qkT", bufs=1)).tile(
            [128, 8, S], BF16)
        vp = ctx.enter_context(tc.tile_pool(name="vp", bufs=1)).tile(
            [128, S // 128, HPC * (DK + 1)], BF16)
        vph = vp.rearrange("p s (h c) -> p s h c", h=HPC)
        heads_t = ctx.enter_context(tc.tile_pool(name="heads", bufs=1)).tile(
            [128, DPC // 128, S], BF16)
        wpool = ctx.enter_context(tc.tile_pool(name="w", bufs=1))
        w_qk = wpool.tile([128, 2, D // 128, DPC], BF16)
        wv_t = wpool.tile([128, D // 128, DPC], BF16)
        wo_t = wpool.tile([128, DPC // 128, D], BF16)

        xts_pool = ctx.enter_context(tc.tile_pool(name="xts", bufs=2))
        e_pool = ctx.enter_context(tc.tile_pool(name="e", bufs=6))
        tmp = ctx.enter_context(tc.tile_pool(name="tmp", bufs=2))
        norm_pool = ctx.enter_context(tc.tile_pool(name="norm", bufs=2))
        ysb_pool = ctx.enter_context(tc.tile_pool(name="ysb", bufs=2))

        ps_s = ctx.enter_context(tc.tile_pool(name="ps_s", bufs=2, space="PSUM"))
        ps_o = ctx.enter_context(tc.tile_pool(name="ps_o", bufs=2, space="PSUM"))
        ps_j = ctx.enter_context(tc.tile_pool(name="ps_j", bufs=2, space="PSUM"))

        # ---- DMA prefetch: host pre-packed SBUF layouts, contiguous DMAs ----
        xts_tiles = {}

        def load_x(sc, eng):
            t = xts_pool.tile([128, D // 128, 512], BF16, name="xts")
            eng.dma_start(out=t, in_=xTc[sc])
            xts_tiles[sc] = t

        nc.scalar.dma_start(out=w_qk[:, 1], in_=wqk_in[:, 1])
        load_x(0, nc.sync)
        nc.gpsimd.dma_start(out=w_qk[:, 0], in_=wqk_in[:, 0])
        nc.gpsimd.dma_start(out=p_swap, in_=psw.bitcast(F32R))
        nc.scalar.dma_start(out=cbig, in_=cst)
        nc.gpsimd.dma_start(out=sbig, in_=sst)
        load_x(1, nc.sync)
        nc.sync.dma_start(out=wv_t, in_=wvc)
        nc.gpsimd.dma_start(out=wo_t, in_=woc)

        # ones column of the V layout (denominator trick)
        nc.scalar.copy(vph[:, :, :, DK:DK + 1],
                       ones_col.to_broadcast((128, S // 128, HPC, 1)))

        # ---- emission units (generators; one `yield` ~ 2 matmuls of PE) ----
        def gen_qk_unit(sc, qk, et, dense=False):
            ssl = bass.ts(sc, 512)
            if dense:
                p_t = ps_s.tile([128, 512], F32, name="p_t", tag="sc_t")
            else:
                p_t = ps_j.tile([128, 512], F32, name="p_t", tag="j")
            xts = xts_tiles[sc]
            for dc in range(D // 128):
                nc.tensor.matmul(
                    p_t, w_qk[:, qk, dc, et * 128:(et + 1) * 128], xts[:, dc, :],
                    start=(dc == 0), stop=(dc == D // 128 - 1))
                if dc % 2 == 1 and dc < 7:
                    yield
            qt_sb = tmp.tile([128, 512], F32R, name="qt_sb")
            nc.vector.tensor_copy(qt_sb, p_t)
            yield
            sw = ps_j.tile([128, 512], F32, name="sw", tag="j")
            nc.tensor.matmul(sw, p_swap, qt_sb, start=True, stop=True)
            g1 = tmp.tile([128, 512], F32, name="g1")
            nc.vector.tensor_mul(g1, qt_sb.bitcast(F32), cbig[:, ssl])
            d1 = tmp.tile([128, 512], F32, name="d1")
            nc.vector.tensor_mul(d1, sw, sbig[:, ssl])
            nc.vector.tensor_add(qkT[:, qk * 4 + et, ssl], g1, d1)
            yield

        def gen_v_unit(sc, st4, dense=False):
            if dense:
                p_t = ps_s.tile([128, 512], F32, name="pv_t", tag="sc_t")
            else:
                p_t = ps_j.tile([128, 512], F32, name="pv_t", tag="j")
            xts = xts_tiles[sc]
            for dc in range(D // 128):
                nc.tensor.matmul(
                    p_t, xts[:, dc, st4 * 128:(st4 + 1) * 128], wv_t[:, dc, :],
                    start=(dc == 0), stop=(dc == D // 128 - 1))
                if dc % 2 == 1 and dc < 7:
                    yield
            nc.vector.tensor_copy(
                vph[:, sc * 4 + st4, :, 0:DK],
                p_t.rearrange("p (h c) -> p h c", h=HPC))
            yield

        def gen_oproj_unit(qc, st4, nb):
            st = qc * 4 + st4
            py = ps_j.tile([128, 512], F32, name="py", tag="j")
            for dc in range(DPC // 128):
                nc.tensor.matmul(
                    py, heads_t[:, dc, st * 128:(st + 1) * 128],
                    wo_t[:, dc, bass.ts(nb, 512)],
                    start=(dc == 0), stop=(dc == DPC // 128 - 1))
                if dc == 1:
                    yield
            y_sb = ysb_pool.tile([128, 512], F32, name="y_sb")
            nc.vector.tensor_copy(y_sb, py)
            nc.sync.dma_start(
                out=y[st * 128:(st + 1) * 128, bass.ts(nb, 512)], in_=y_sb)
            yield

        # ---- filler queue ----
        filler_q = []   # entries: (key, generator)

        def push_proj(sc):
            for et in range(4):
                filler_q.append((("p", sc), gen_qk_unit(sc, 1, et)))
                filler_q.append((("p", sc), gen_qk_unit(sc, 0, et)))
            for st4 in range(4):
                filler_q.append((("p", sc), gen_v_unit(sc, st4)))

        def push_oproj(qc):
            for st4 in range(4):
                for nb in range(D // 512):
                    filler_q.append((("o", qc), gen_oproj_unit(qc, st4, nb)))

        def pull(n):
            got = 0
            while got < n and filler_q:
                try:
                    next(filler_q[0][1])
                    got += 1
                except StopIteration:
                    filler_q.pop(0)
            return got

        def drain_key(key):
            while filler_q and any(k == key for k, _ in filler_q):
                try:
                    next(filler_q[0][1])
                except StopIteration:
                    filler_q.pop(0)

        # ---- attention for one 512-query chunk ----
        def attn(qc):
            qsl = bass.ts(qc, 512)
            n_kt = 4 * qc + 4
            kt_order = list(range(4 * qc, 4 * qc + 4)) + list(range(0, 4 * qc))
            act_t, pe_t = 0.0, 0.0
            pending_norm = [None]
            for hp in range(HPC // 2):
                hA, hB = 2 * hp, 2 * hp + 1
                o_ts = [ps_o.tile([DK + 1, 512], F32, name=f"o{ab}", tag="o")
                        for ab in "AB"]

                def emit_scores(kt):
                    diag = (kt // 4 == qc)
                    co = 128 * (kt % 4) if diag else 0
                    n = 512 - co
                    ktsl = bass.ts(kt, 128)
                    q_ap = bass.ds(qc * 512 + co, n)
                    sc_t = ps_s.tile([128, 1024], F32, name="sc_t", tag="sc_t")
                    for i, (ro, tp) in enumerate(((0, (0, 0)), (64, (64, 0)))):
                        nc.tensor.matmul(
                            sc_t[:, i * 512:i * 512 + n],
                            qkT[ro:ro + 64, 4 + hp, ktsl],
                            qkT[ro:ro + 64, hp, q_ap],
                            start=True, stop=True, tile_position=tp)
                    e_t = e_pool.tile([128, 1024], BF16, name="e_t")
                    nc.scalar.activation(e_t, sc_t, AF.Exp,
                                         scale=float(1.0 / np.sqrt(DK)))
                    if diag:
                        for i in range(2):
                            nc.gpsimd.affine_select(
                                out=e_t[:, i * 512:i * 512 + 128],
                                in_=e_t[:, i * 512:i * 512 + 128],
                                pattern=[[1, 128]], base=0,
                                channel_multiplier=-1,
                                compare_op=mybir.AluOpType.is_ge, fill=0.0)
                    return kt, e_t, n

                def emit_av(kt, e_t, n, start, stop):
                    co = 512 - n
                    for i, h in enumerate((hA, hB)):
                        nc.tensor.matmul(
                            o_ts[i][:, co:512],
                            vp[:, kt, h * (DK + 1):(h + 1) * (DK + 1)],
                            e_t[:, i * 512:i * 512 + n],
                            start=start, stop=stop)

                pend = []
                n_av = 0
                for idx, kt in enumerate(kt_order):
                    cur = emit_scores(kt)
                    if idx == 0:
                        pull(3)
                    if idx == 2 and pending_norm[0] is not None:
                        pending_norm[0]()
                        pending_norm[0] = None
                    act_t += _EXP_NS
                    pe_t += _mm_ns(cur[2])
                    while pe_t < act_t and filler_q:
                        pe_t += _YIELD_NS * max(1, pull(1))
                        if not filler_q:
                            break
                    if len(pend) == 4:
                        p = pend.pop(0)
                        emit_av(p[0], p[1], p[2],
                                start=(n_av == 0), stop=False)
                        n_av += 1
                        pe_t += 2 * _mm_ns(p[2])
                    pend.append(cur)
                for p in pend:
                    emit_av(p[0], p[1], p[2], start=(n_av == 0),
                            stop=(n_av == n_kt - 1))
                    n_av += 1
                    pe_t += 2 * _mm_ns(p[2])
                if n_kt <= 2 and pending_norm[0] is not None:
                    pending_norm[0]()
                    pending_norm[0] = None

                # normalize: o / denominator, per head, straight out of PSUM
                # part 1: copy o out of PSUM immediately (frees the o slots
                # for the next head pair) and take reciprocals
                obA = norm_pool.tile([DK, 512], F32, name="obA")
                obB = norm_pool.tile([DK, 512], F32, name="obB")
                nc.vector.tensor_copy(obA, o_ts[0][0:DK, :])
                nc.vector.tensor_copy(obB, o_ts[1][0:DK, :])
                dsbA = norm_pool.tile([1, 512], F32, name="dsbA", bufs=1)
                dsbB = norm_pool.tile([1, 512], F32, name="dsbB", bufs=1)
                nc.scalar.copy(dsbA, o_ts[0][DK:DK + 1, :])
                nc.scalar.copy(dsbB, o_ts[1][DK:DK + 1, :])
                recipA = norm_pool.tile([1, 512], F32, name="recipA", bufs=1)
                recipB = norm_pool.tile([1, 512], F32, name="recipB", bufs=1)
                nc.vector.reciprocal_approx_fast(recipA, dsbA)
                nc.vector.reciprocal_approx_fast(recipB, dsbB)

                # part 2: broadcast the reciprocals down 64 partitions with a
                # K=1 PE outer product (keeps gpsimd free for the causal masks)
                def finish_norm(hp=hp, obA=obA, obB=obB,
                                recipA=recipA, recipB=recipB):
                    rbA = norm_pool.tile([DK, 512], F32, name="rbA")
                    rbB = norm_pool.tile([DK, 512], F32, name="rbB")
                    nc.gpsimd.partition_broadcast(rbA, recipA)
                    nc.gpsimd.partition_broadcast(rbB, recipB)
                    nc.vector.tensor_mul(heads_t[0:DK, hp, qsl], obA, rbA)
                    hnB = norm_pool.tile([DK, 512], BF16, name="hnB")
                    nc.vector.tensor_mul(hnB, obB, rbB)
                    nc.sync.dma_start(out=heads_t[DK:2 * DK, hp, qsl], in_=hnB)
                pending_norm[0] = finish_norm
            if pending_norm[0] is not None:
                pending_norm[0]()
                pending_norm[0] = None

        # ---- fused schedule ----
        # dense proj(0) through the wide ps_s slots (attention not live yet)
        for et in range(4):
            for g in (gen_qk_unit(0, 1, et, dense=True),
                      gen_qk_unit(0, 0, et, dense=True)):
                for _ in g:
                    pass
        for st4 in range(4):
            for _ in gen_v_unit(0, st4, dense=True):
                pass

        push_proj(1)
        attn(0)
        push_proj(2)
        load_x(2, nc.sync)
        drain_key(("p", 1))
        attn(1)
        push_proj(3)
        push_oproj(0)
        load_x(3, nc.sync)
        drain_key(("p", 2))
        attn(2)
        push_oproj(1)
        push_oproj(2)
        drain_key(("p", 3))
        attn(3)
        push_oproj(3)
        while filler_q:
            pull(1000)

        if dbg is not None:
            nc.sync.dma_start(out=dbg["cs_dump"][0], in_=cbig)
            nc.sync.dma_start(out=dbg["cs_dump"][1], in_=sbig)
            nc.sync.dma_start(out=dbg["qk_dump"], in_=qkT)
            nc.sync.dma_start(out=dbg["vp_dump"], in_=vp)
            nc.sync.dma_start(out=dbg["heads_dump"], in_=heads_t)


def _host_tables(pos_np):
    inv_freq = 1.0 / (THETA ** (np.arange(HALF, dtype=np.float32).astype(np.float64) * 2.0 / DK))
    ang = pos_np.astype(np.float64)[None, :] * inv_freq[:, None]   # [32, S]
    c = np.cos(ang).astype(np.float32)
    s = np.sin(ang).astype(np.float32)
    cbig = np.tile(np.concatenate([c, c], axis=0), (2, 1))         # [128, S]
    sbig = np.tile(np.concatenate([-s, s], axis=0), (2, 1))
    return np.ascontiguousarray(cbig), np.ascontiguousarray(sbig)


_program_cache = None


def _get_program():
    global _program_cache
    if _program_cache is None:
        _program_cache = _build_program()
    return _program_cache


# dk permutation: evens then odds within each head's 64 dims
_PERM64 = np.concatenate([np.arange(0, DK, 2), np.arange(1, DK, 2)])


def _chunk_pack(wT):
    # [D_in, E] -> [128, D_in//128, E] partition-major chunk layout
    d, e = wT.shape
    return np.ascontiguousarray(wT.reshape(d // 128, 128, e).transpose(1, 0, 2))


def _host_pswap():
    p = np.zeros((128, 128), dtype=np.float32)
    for blk in range(4):
        s = (blk ^ 1) * 32
        for i in range(32):
            p[blk * 32 + i, s + i] = 1.0
    return p


def _make_in_maps(x, Wq, Wk, Wv, Wo, pos_np):
    bf = ml_dtypes.bfloat16
    cbig, sbig = _host_tables(pos_np)
    psw = _host_pswap()
    in_maps = []
    for c in range(N_CORES):
        b, hg = c // 2, c % 2
        rows = hg * DPC + np.concatenate(
            [h * DK + _PERM64 for h in range(HPC)])
        xT = x[b].T.astype(bf)                       # [D, S]
        xtc = np.ascontiguousarray(
            xT.reshape(D // 128, 128, 4, 512).transpose(2, 1, 0, 3))
        wq = _chunk_pack(Wq[rows, :].T.astype(bf))   # [128, 8, 512]
        wk = _chunk_pack(Wk[rows, :].T.astype(bf))
        wqk = np.ascontiguousarray(np.stack([wq, wk], axis=1))
        in_maps.append({
            "xTc": xtc,
            "wqk": wqk,
            "wvc": _chunk_pack(Wv[hg * DPC:(hg + 1) * DPC, :].T.astype(bf)),
            "woc": _chunk_pack(Wo[:, hg * DPC:(hg + 1) * DPC].T.astype(bf)),
            "cst": cbig,
            "sst": sbig,
            "psw": psw,
        })
    return in_maps


def kernel(x, Wq, Wk, Wv, Wo, token_positions):
    x = np.asarray(x, dtype=np.float32)
    Wq = np.asarray(Wq, dtype=np.float32)
    Wk = np.asarray(Wk, dtype=np.float32)
    Wv = np.asarray(Wv, dtype=np.float32)
    Wo = np.asarray(Wo, dtype=np.float32)
    pos_np = np.ascontiguousarray(np.asarray(token_positions, dtype=np.int32))

    nc = _get_program()
    in_maps = _make_in_maps(x, Wq, Wk, Wv, Wo, pos_np)
    res = run_bass_kernel_spmd(nc, in_maps, list(range(N_CORES)))
    out = np.empty((B, S, D), dtype=np.float32)
    for b in range(B):
        out[b] = res.results[2 * b]["y"] + res.results[2 * b + 1]["y"]
    return out
